# revision 2
# baseline (speedup 1.0000x reference)
"""DecorConv TRN2 kernel: unfold(3x3) -> decor matmul -> norm -> 1x1 conv.

Math restructure (per reference):
    u = unfold(input)                         # (B, 576, 1024)
    d = Wd^T @ u  (per batch)                 # only needed for statistics!
    norm[n] = sqrt(mean u[n]^2) / (sqrt(mean d[n]^2) + 1e-8)
    out = Wc @ diag(norm) @ d = (Wc @ diag(norm) @ Wd^T) @ u

So the 576x576 decor matmul is used ONLY to accumulate sum(d^2) straight out
of PSUM (d is never stored), and the output matmul contracts the small fused
weight  combined = Wc @ diag(norm) @ Wd^T  (256x576) against u.

Sharding: data-parallel over batch, 8 batches/core on 8 cores. The per-channel
statistics (2x576 floats) are AllReduce'd across cores; everything else is
core-local. Weights are replicated.

On-chip layout: u rows use the permuted order r = o*64 + c (offset-major,
o = kh*3+kw), so each 64-row block of u is one shifted window copy of the
padded input. decor_weight rows are permuted identically on the host, which
leaves the result d (and everything downstream) in canonical channel order.
The u-channel statistics are un-permuted by a strided DMA when writing the
AllReduce bounce buffer.

Matmuls run in float32r (TF32-like: ~2.5e-4 max rel err, 4x faster than
native fp32 on the PE).
"""

import numpy as np

import concourse.mybir as mybir
import concourse.tile as tile
from concourse import bacc
from concourse.bass_utils import run_bass_kernel_spmd

F32 = mybir.dt.float32
F32R = mybir.dt.float32r
AF = mybir.ActivationFunctionType
ALU = mybir.AluOpType
AX = mybir.AxisListType

NCORES = 8
B, C, H, W = 64, 64, 32, 32
BL = B // NCORES            # batches per core
L = H * W                   # 1024
HP = WP = 34                # padded spatial
LP = HP * WP                # 1156
N = 576                     # C*9 contraction channels
O = 256                     # output channels
KT = [128, 128, 128, 128, 64]   # K-tiling of 576
NT = 5

TRACE = False               # set True to collect an NTFF profile / exec time
_CACHE = {}


def _build():
    nc = bacc.Bacc("TRN2", target_bir_lowering=False, debug=False,
                   num_devices=NCORES)

    xpad = nc.dram_tensor("xpad", [BL, C, LP], F32R, kind="ExternalInput").ap()
    wd = nc.dram_tensor("wd", [N, N], F32R, kind="ExternalInput").ap()      # [r, m]
    wdt = nc.dram_tensor("wdt", [N, N], F32R, kind="ExternalInput").ap()    # [m, r]
    wct = nc.dram_tensor("wct", [N, O], F32R, kind="ExternalInput").ap()    # [m, o]
    out = nc.dram_tensor("out", [BL, O, L], F32, kind="ExternalOutput").ap()

    with tile.TileContext(nc) as tc:
        with (
            tc.tile_pool(name="wpool", bufs=1) as wpool,
            tc.tile_pool(name="upool", bufs=4) as upool,
            tc.tile_pool(name="iopool", bufs=2) as iopool,
            tc.tile_pool(name="spool", bufs=1) as spool,
            tc.tile_pool(name="tpool", bufs=3) as tpool,
            tc.tile_pool(name="psum", bufs=1, space="PSUM") as psum,
            tc.tile_pool(name="dram", bufs=1, space="DRAM") as dram,
        ):
            # ---- weight loads ----
            wd_t, wdt_t, wct_t = [], [], []
            off = 0
            for k, kw_ in enumerate(KT):
                wdk = wpool.tile([128, N], F32R, name=f"wd{k}", tag=f"wd{k}")
                nc.sync.dma_start(out=wdk[0:kw_, :], in_=wd[off:off + kw_, :])
                wd_t.append(wdk)
                wdtk = wpool.tile([128, N], F32R, name=f"wdt{k}", tag=f"wdt{k}")
                nc.sync.dma_start(out=wdtk[0:kw_, :], in_=wdt[off:off + kw_, :])
                wdt_t.append(wdtk)
                wck = wpool.tile([128, O], F32R, name=f"wct{k}", tag=f"wct{k}")
                nc.sync.dma_start(out=wck[0:kw_, :], in_=wct[off:off + kw_, :])
                wct_t.append(wck)
                off += kw_

            # ---- stats partials ----
            usq_p = spool.tile([128, NT * BL], F32, name="usq_p")
            dsq_p = spool.tile([128, NT * BL], F32, name="dsq_p")
            nc.vector.memset(usq_p, 0.0)
            nc.vector.memset(dsq_p, 0.0)
            zero64 = spool.tile([1, 64], F32, name="zero64")
            nc.vector.memset(zero64, 0.0)

            u_tiles = {}

            def build_u(b):
                """DMA the padded batch in, carve the 9 shifted windows."""
                inp = iopool.tile([128, LP], F32R, name=f"inp{b}", tag="inpad")
                nc.sync.dma_start(out=inp[0:64, :], in_=xpad[b])
                nc.sync.dma_start(out=inp[64:128, :], in_=xpad[b])
                ipv = inp.rearrange("p (h w) -> p h w", h=HP, w=WP)
                for t in range(NT):
                    ut = upool.tile([128, L], F32R, name=f"u{b}_{t}", tag=f"u{t}")
                    uv = ut.rearrange("p (h w) -> p h w", h=H, w=W)
                    o0 = 2 * t
                    kh, kw_ = o0 // 3, o0 % 3
                    nc.vector.tensor_copy(
                        uv[0:64], ipv[0:64, kh:kh + H, kw_:kw_ + W])
                    if t < 4:
                        o1 = 2 * t + 1
                        kh, kw_ = o1 // 3, o1 % 3
                        nc.vector.tensor_copy(
                            uv[64:128], ipv[64:128, kh:kh + H, kw_:kw_ + W])
                    u_tiles[(b, t)] = ut

            # ================= phase 1: d^2 / u^2 statistics =================
            for b in range(BL):
                build_u(b)
                for mt in range(NT):
                    mw = KT[mt]
                    ms = 128 * mt
                    pd = psum.tile([128, L], F32, name=f"pd{b}_{mt}", tag="d",
                                   bufs=2)
                    for k in range(NT):
                        kp = KT[k]
                        for ns in (0, 512):
                            nc.tensor.matmul(
                                pd[0:mw, ns:ns + 512],
                                wd_t[k][0:kp, ms:ms + mw],
                                u_tiles[(b, k)][0:kp, ns:ns + 512],
                                start=(k == 0), stop=(k == 4))
                    dtr = tpool.tile([128, L], F32, name=f"dtr{b}_{mt}",
                                     tag="sqtrash")
                    nc.scalar.activation(dtr[0:mw, :], pd[0:mw, :], AF.Square,
                                         accum_out=dsq_p[0:mw, mt * BL + b:mt * BL + b + 1])
                for t in range(NT):
                    kp = KT[t]
                    utr = tpool.tile([128, L], F32, name=f"utr{b}_{t}",
                                     tag="sqtrash")
                    nc.scalar.activation(utr[0:kp, :], u_tiles[(b, t)][0:kp, :],
                                         AF.Square,
                                         accum_out=usq_p[0:kp, t * BL + b:t * BL + b + 1])

            # ================= stats reduce + AllReduce =================
            usq_l = spool.tile([128, NT], F32, name="usq_l")
            dsq_l = spool.tile([128, NT], F32, name="dsq_l")
            nc.vector.reduce_sum(
                usq_l, usq_p.rearrange("p (t b) -> p t b", b=BL), axis=AX.X)
            nc.vector.reduce_sum(
                dsq_l, dsq_p.rearrange("p (t b) -> p t b", b=BL), axis=AX.X)

            ar_in = dram.tile([1280], F32, name="ar_in")
            ar_out = dram.tile([1280], F32, name="ar_out", addr_space="Shared")
            # u-stats: permute r = o*64+c  ->  canonical n = c*9+o on the way out
            ar_u = ar_in[0:N].rearrange("(c o) -> c o", o=9)
            nc.sync.dma_start(out=ar_u[:, 0:9:2], in_=usq_l[0:64, 0:5])
            nc.sync.dma_start(out=ar_u[:, 1:9:2], in_=usq_l[64:128, 0:4])
            nc.sync.dma_start(out=ar_in[N:640], in_=zero64)
            # d-stats: canonical m = 128*t + p, padded [640] layout
            nc.sync.dma_start(
                out=ar_in[640:1280].rearrange("(t p) -> p t", p=128), in_=dsq_l)
            nc.gpsimd.collective_compute(
                "AllReduce", ALU.add,
                replica_groups=[list(range(NCORES))],
                ins=[ar_in.opt()], outs=[ar_out.opt()])

            usq_s = spool.tile([128, NT], F32, name="usq_s")
            dsq_s = spool.tile([128, NT], F32, name="dsq_s")
            nc.sync.dma_start(
                out=usq_s, in_=ar_out[0:640].rearrange("(t p) -> p t", p=128))
            nc.sync.dma_start(
                out=dsq_s, in_=ar_out[640:1280].rearrange("(t p) -> p t", p=128))

            # norm = sqrt(mean u^2) / (sqrt(mean d^2) + 1e-8)
            s_u = spool.tile([128, NT], F32, name="s_u")
            s_d = spool.tile([128, NT], F32, name="s_d")
            inv_bl = 1.0 / (B * L)
            nc.scalar.activation(s_u, usq_s, AF.Sqrt, scale=inv_bl)
            nc.scalar.activation(s_d, dsq_s, AF.Sqrt, scale=inv_bl)
            sde = spool.tile([128, NT], F32, name="sde")
            nc.vector.tensor_scalar_add(sde, s_d, 1e-8)
            rin = spool.tile([128, NT], F32, name="rin")
            nc.vector.reciprocal(rin, sde)
            normt = spool.tile([128, NT], F32, name="normt")
            nc.vector.tensor_mul(normt, s_u, rin)

            # combined^T[r, o] = sum_m wd_perm[r, m] * norm[m] * Wc[o, m]
            wds_t, cmb_t = [], []
            for k in range(NT):
                kp = KT[k]
                wds = wpool.tile([128, N], F32R, name=f"wds{k}", tag=f"wds{k}")
                nc.vector.tensor_scalar_mul(
                    wds[0:kp, :], wdt_t[k][0:kp, :], normt[0:kp, k:k + 1])
                wds_t.append(wds)
            for rt in range(NT):
                rw = KT[rt]
                rs = 128 * rt
                pc = psum.tile([128, O], F32, name=f"pc{rt}", tag="c", bufs=1)
                for k in range(NT):
                    kp = KT[k]
                    nc.tensor.matmul(
                        pc[0:rw, :],
                        wds_t[k][0:kp, rs:rs + rw],
                        wct_t[k][0:kp, :],
                        start=(k == 0), stop=(k == 4))
                cmb = wpool.tile([128, O], F32R, name=f"cmb{rt}", tag=f"cmb{rt}")
                nc.scalar.copy(cmb[0:rw, :], pc[0:rw, :])
                cmb_t.append(cmb)

            # ================= phase 2: out = combined^T.T @ u =================
            for b in list(range(BL - 1, BL - 5, -1)) + list(range(BL - 5, -1, -1)):
                if b <= BL - 5:
                    # evicted during phase 1 (only the last 4 batches stay
                    # resident in the bufs=4 u slots) -> rebuild
                    build_u(b)
                for ot in range(2):
                    for nt_ in range(2):
                        po = psum.tile([128, 512], F32, name=f"po{b}_{ot}_{nt_}",
                                       tag="o", bufs=3)
                        for rt in range(NT):
                            rw = KT[rt]
                            nc.tensor.matmul(
                                po,
                                cmb_t[rt][0:rw, 128 * ot:128 * (ot + 1)],
                                u_tiles[(b, rt)][0:rw, 512 * nt_:512 * (nt_ + 1)],
                                start=(rt == 0), stop=(rt == 4))
                        osb = tpool.tile([128, 512], F32, name=f"osb{b}_{ot}_{nt_}",
                                         tag="osb", bufs=4)
                        if (ot + nt_) % 2 == 0:
                            nc.scalar.copy(osb, po)
                        else:
                            nc.vector.tensor_copy(osb, po)
                        nc.sync.dma_start(
                            out=out[b, 128 * ot:128 * (ot + 1),
                                    512 * nt_:512 * (nt_ + 1)],
                            in_=osb)

    nc.compile()
    return nc


def kernel(input, decor_weight, conv_weight):
    input = np.asarray(input, dtype=np.float32)
    decor_weight = np.asarray(decor_weight, dtype=np.float32)
    conv_weight = np.asarray(conv_weight, dtype=np.float32)

    if "nc" not in _CACHE:
        _CACHE["nc"] = _build()
    nc = _CACHE["nc"]

    # host-side weight prep: permute decor rows to the on-chip u-row order
    r = np.arange(N)
    perm = (r % 64) * 9 + (r // 64)          # n(r): canonical row for u-row r
    wd_perm = np.ascontiguousarray(decor_weight[perm, :])       # [r, m]
    wd_perm_t = np.ascontiguousarray(wd_perm.T)                 # [m, r]
    wc_t = np.ascontiguousarray(conv_weight.T)                  # [m, o]

    xp = np.zeros((B, C, HP, WP), dtype=np.float32)
    xp[:, :, 1:1 + H, 1:1 + W] = input
    xp = xp.reshape(B, C, LP)

    in_maps = []
    for i in range(NCORES):
        in_maps.append({
            "xpad": np.ascontiguousarray(xp[i * BL:(i + 1) * BL]),
            "wd": wd_perm,
            "wdt": wd_perm_t,
            "wct": wc_t,
        })

    res = run_bass_kernel_spmd(nc, in_maps, core_ids=list(range(NCORES)),
                               trace=TRACE)
    _CACHE["last_result"] = res

    full = np.concatenate([res.results[i]["out"] for i in range(NCORES)], axis=0)
    return full.reshape(B, O, H, W).astype(np.float32)


# revision 3
# speedup vs baseline: 1.1065x; 1.1065x over previous
"""DecorConv TRN2 kernel: unfold(3x3) -> decor matmul -> norm -> 1x1 conv.

Math restructure (per reference):
    u = unfold(input)                         # (B, 576, 1024)
    d = Wd^T @ u  (per batch)                 # only needed for statistics!
    norm[n] = sqrt(mean u[n]^2) / (sqrt(mean d[n]^2) + 1e-8)
    out = Wc @ diag(norm) @ d = (Wc @ diag(norm) @ Wd^T) @ u

So the 576x576 decor matmul is used ONLY to accumulate sum(d^2) straight out
of PSUM (d is never stored), and the output matmul contracts the small fused
weight  combined = Wc @ diag(norm) @ Wd^T  (256x576) against u.

Sharding: data-parallel over batch, 8 batches/core on 8 cores. The per-channel
statistics (2x576 floats) are AllReduce'd across cores; everything else is
core-local. Weights are replicated.

On-chip layout: u rows use the permuted order r = o*64 + c (offset-major,
o = kh*3+kw), so each 64-row block of u is one shifted window copy of the
padded input. decor_weight rows are permuted identically on the host, which
leaves d (and everything downstream) in canonical channel order. The
u-channel statistics are un-permuted by a strided DMA when writing the
AllReduce bounce buffer.

dtypes: the two big matmul passes run in fp16 (11-bit mantissa, products are
exact in the fp32 PSUM accumulate; ~2x faster than fp32r on the PE because
the weight loads pipeline). The small norm-scaled weight-product matmul runs
in float32r for precision. A column-shifted second copy of the padded input
keeps every window copy 4-byte aligned so the DVE runs them in 4x mode.
"""

import numpy as np

import concourse.mybir as mybir
import concourse.tile as tile
from concourse import bacc
from concourse.bass_utils import run_bass_kernel_spmd

F32 = mybir.dt.float32
F32R = mybir.dt.float32r
FP16 = mybir.dt.float16
AF = mybir.ActivationFunctionType
ALU = mybir.AluOpType
AX = mybir.AxisListType

NCORES = 8
B, C, H, W = 64, 64, 32, 32
BL = B // NCORES            # batches per core
L = H * W                   # 1024
HP = WP = 34                # padded spatial
LP = HP * WP                # 1156
N = 576                     # C*9 contraction channels
O = 256                     # output channels
KT = [128, 128, 128, 128, 64]   # K-tiling of 576
NT = 5

TRACE = False               # set True to collect an NTFF profile / exec time
_CACHE = {}


def _build():
    nc = bacc.Bacc("TRN2", target_bir_lowering=False, debug=False,
                   num_devices=NCORES)

    xpa = nc.dram_tensor("xpa", [BL, C, LP], FP16, kind="ExternalInput").ap()
    xpb = nc.dram_tensor("xpb", [BL, C, LP], FP16, kind="ExternalInput").ap()
    wd = nc.dram_tensor("wd", [N, N], FP16, kind="ExternalInput").ap()       # [r, m]
    wdt = nc.dram_tensor("wdt", [N, N], F32R, kind="ExternalInput").ap()     # [m, r]
    wct = nc.dram_tensor("wct", [N, O], F32R, kind="ExternalInput").ap()     # [m, o]
    out = nc.dram_tensor("out", [BL, O, L], F32, kind="ExternalOutput").ap()

    with tile.TileContext(nc) as tc:
        with (
            tc.tile_pool(name="wpool", bufs=1) as wpool,
            tc.tile_pool(name="upool", bufs=BL) as upool,
            tc.tile_pool(name="iopool", bufs=2) as iopool,
            tc.tile_pool(name="spool", bufs=1) as spool,
            tc.tile_pool(name="tpool", bufs=3) as tpool,
            tc.tile_pool(name="psum", bufs=1, space="PSUM") as psum,
            tc.tile_pool(name="dram", bufs=1, space="DRAM") as dram,
        ):
            # ---- weight loads ----
            wd_t, wdt_t, wct_t = [], [], []
            off = 0
            for k, kw_ in enumerate(KT):
                wdk = wpool.tile([128, N], FP16, name=f"wd{k}", tag=f"wd{k}")
                nc.sync.dma_start(out=wdk[0:kw_, :], in_=wd[off:off + kw_, :])
                wd_t.append(wdk)
                wdtk = wpool.tile([128, N], F32R, name=f"wdt{k}", tag=f"wdt{k}")
                nc.sync.dma_start(out=wdtk[0:kw_, :], in_=wdt[off:off + kw_, :])
                wdt_t.append(wdtk)
                wck = wpool.tile([128, O], F32R, name=f"wct{k}", tag=f"wct{k}")
                nc.sync.dma_start(out=wck[0:kw_, :], in_=wct[off:off + kw_, :])
                wct_t.append(wck)
                off += kw_

            # ---- stats partials ----
            usq_p = spool.tile([128, NT * BL], F32, name="usq_p")
            dsq_p = spool.tile([128, NT * BL], F32, name="dsq_p")
            nc.vector.memset(usq_p, 0.0)
            nc.vector.memset(dsq_p, 0.0)
            zero64 = spool.tile([1, 64], F32, name="zero64")
            nc.vector.memset(zero64, 0.0)

            u_tiles = {}

            def build_u(b):
                """DMA the padded batch in, carve the 9 shifted windows.

                Window (kh, kw): reads the kw-aligned copy for kw in {0, 2}
                and the column-shifted copy for kw == 1 so the innermost run
                always starts 4-byte aligned (DVE 4x mode).
                """
                ia = iopool.tile([128, LP], FP16, name=f"ia{b}", tag="ipa")
                nc.sync.dma_start(out=ia[0:64, :], in_=xpa[b])
                nc.sync.dma_start(out=ia[64:128, :], in_=xpa[b])
                ib = iopool.tile([128, LP], FP16, name=f"ib{b}", tag="ipb")
                nc.sync.dma_start(out=ib[0:64, :], in_=xpb[b])
                nc.sync.dma_start(out=ib[64:128, :], in_=xpb[b])
                iav = ia.rearrange("p (h w) -> p h w", h=HP, w=WP)
                ibv = ib.rearrange("p (h w) -> p h w", h=HP, w=WP)

                def src(o, lo, hi):
                    kh, kw_ = o // 3, o % 3
                    v = ibv if kw_ == 1 else iav
                    c0 = kw_ - 1 if kw_ == 1 else kw_
                    return v[lo:hi, kh:kh + H, c0:c0 + W]

                for t in range(NT):
                    ut = upool.tile([128, L], FP16, name=f"u{b}_{t}", tag=f"u{t}")
                    uv = ut.rearrange("p (h w) -> p h w", h=H, w=W)
                    nc.vector.tensor_copy(uv[0:64], src(2 * t, 0, 64))
                    if t < 4:
                        nc.vector.tensor_copy(uv[64:128], src(2 * t + 1, 64, 128))
                    u_tiles[(b, t)] = ut

            # ================= phase 1: d^2 / u^2 statistics =================
            for b in range(BL):
                build_u(b)
                for t in range(NT):
                    kp = KT[t]
                    utr = tpool.tile([128, L], FP16, name=f"utr{b}_{t}",
                                     tag="sqtrash")
                    nc.scalar.activation(utr[0:kp, :], u_tiles[(b, t)][0:kp, :],
                                         AF.Square,
                                         accum_out=usq_p[0:kp, t * BL + b:t * BL + b + 1])
                for mt in range(NT):
                    mw = KT[mt]
                    ms = 128 * mt
                    pd = psum.tile([128, L], F32, name=f"pd{b}_{mt}", tag="d",
                                   bufs=2)
                    for k in range(NT):
                        kp = KT[k]
                        for ns in (0, 512):
                            nc.tensor.matmul(
                                pd[0:mw, ns:ns + 512],
                                wd_t[k][0:kp, ms:ms + mw],
                                u_tiles[(b, k)][0:kp, ns:ns + 512],
                                start=(k == 0), stop=(k == 4))
                    dtr = tpool.tile([128, L], FP16, name=f"dtr{b}_{mt}",
                                     tag="sqtrash")
                    nc.scalar.activation(dtr[0:mw, :], pd[0:mw, :], AF.Square,
                                         accum_out=dsq_p[0:mw, mt * BL + b:mt * BL + b + 1])

            # ================= stats reduce + AllReduce =================
            usq_l = spool.tile([128, NT], F32, name="usq_l")
            dsq_l = spool.tile([128, NT], F32, name="dsq_l")
            nc.vector.reduce_sum(
                usq_l, usq_p.rearrange("p (t b) -> p t b", b=BL), axis=AX.X)
            nc.vector.reduce_sum(
                dsq_l, dsq_p.rearrange("p (t b) -> p t b", b=BL), axis=AX.X)

            ar_in = dram.tile([1280], F32, name="ar_in")
            ar_out = dram.tile([1280], F32, name="ar_out", addr_space="Shared")
            # u-stats: permute r = o*64+c  ->  canonical n = c*9+o on the way out
            ar_u = ar_in[0:N].rearrange("(c o) -> c o", o=9)
            nc.sync.dma_start(out=ar_u[:, 0:9:2], in_=usq_l[0:64, 0:5])
            nc.sync.dma_start(out=ar_u[:, 1:9:2], in_=usq_l[64:128, 0:4])
            nc.sync.dma_start(out=ar_in[N:640], in_=zero64)
            # d-stats: canonical m = 128*t + p, padded [640] layout
            nc.sync.dma_start(
                out=ar_in[640:1280].rearrange("(t p) -> p t", p=128), in_=dsq_l)
            nc.gpsimd.collective_compute(
                "AllReduce", ALU.add,
                replica_groups=[list(range(NCORES))],
                ins=[ar_in.opt()], outs=[ar_out.opt()])

            usq_s = spool.tile([128, NT], F32, name="usq_s")
            dsq_s = spool.tile([128, NT], F32, name="dsq_s")
            nc.sync.dma_start(
                out=usq_s, in_=ar_out[0:640].rearrange("(t p) -> p t", p=128))
            nc.sync.dma_start(
                out=dsq_s, in_=ar_out[640:1280].rearrange("(t p) -> p t", p=128))

            # norm = sqrt(mean u^2) / (sqrt(mean d^2) + 1e-8)
            s_u = spool.tile([128, NT], F32, name="s_u")
            s_d = spool.tile([128, NT], F32, name="s_d")
            inv_bl = 1.0 / (B * L)
            nc.scalar.activation(s_u, usq_s, AF.Sqrt, scale=inv_bl)
            nc.scalar.activation(s_d, dsq_s, AF.Sqrt, scale=inv_bl)
            sde = spool.tile([128, NT], F32, name="sde")
            nc.vector.tensor_scalar_add(sde, s_d, 1e-8)
            rin = spool.tile([128, NT], F32, name="rin")
            nc.vector.reciprocal(rin, sde)
            normt = spool.tile([128, NT], F32, name="normt")
            nc.vector.tensor_mul(normt, s_u, rin)

            # combined^T[r, o] = sum_m wd_perm[r, m] * norm[m] * Wc[o, m]
            wds_t, cmb_t = [], []
            for k in range(NT):
                kp = KT[k]
                wds = wpool.tile([128, N], F32R, name=f"wds{k}", tag=f"wds{k}")
                nc.vector.tensor_scalar_mul(
                    wds[0:kp, :], wdt_t[k][0:kp, :], normt[0:kp, k:k + 1])
                wds_t.append(wds)
            for rt in range(NT):
                rw = KT[rt]
                rs = 128 * rt
                pc = psum.tile([128, O], F32, name=f"pc{rt}", tag="c", bufs=1)
                for k in range(NT):
                    kp = KT[k]
                    nc.tensor.matmul(
                        pc[0:rw, :],
                        wds_t[k][0:kp, rs:rs + rw],
                        wct_t[k][0:kp, :],
                        start=(k == 0), stop=(k == 4))
                cmb = wpool.tile([128, O], FP16, name=f"cmb{rt}", tag=f"cmb{rt}")
                nc.scalar.copy(cmb[0:rw, :], pc[0:rw, :])
                cmb_t.append(cmb)

            # ================= phase 2: out = combined^T.T @ u =================
            for b in range(BL - 1, -1, -1):
                for ot in range(2):
                    for nt_ in range(2):
                        po = psum.tile([128, 512], F32, name=f"po{b}_{ot}_{nt_}",
                                       tag="o", bufs=3)
                        for rt in range(NT):
                            rw = KT[rt]
                            nc.tensor.matmul(
                                po,
                                cmb_t[rt][0:rw, 128 * ot:128 * (ot + 1)],
                                u_tiles[(b, rt)][0:rw, 512 * nt_:512 * (nt_ + 1)],
                                start=(rt == 0), stop=(rt == 4))
                        osb = tpool.tile([128, 512], F32, name=f"osb{b}_{ot}_{nt_}",
                                         tag="osb", bufs=4)
                        if (ot + nt_) % 2 == 0:
                            nc.scalar.copy(osb, po)
                        else:
                            nc.vector.tensor_copy(osb, po)
                        nc.sync.dma_start(
                            out=out[b, 128 * ot:128 * (ot + 1),
                                    512 * nt_:512 * (nt_ + 1)],
                            in_=osb)

    nc.compile()
    return nc


def kernel(input, decor_weight, conv_weight):
    input = np.asarray(input, dtype=np.float32)
    decor_weight = np.asarray(decor_weight, dtype=np.float32)
    conv_weight = np.asarray(conv_weight, dtype=np.float32)

    if "nc" not in _CACHE:
        _CACHE["nc"] = _build()
    nc = _CACHE["nc"]

    # host-side weight prep: permute decor rows to the on-chip u-row order
    r = np.arange(N)
    perm = (r % 64) * 9 + (r // 64)          # n(r): canonical row for u-row r
    wd_perm = np.ascontiguousarray(decor_weight[perm, :])       # [r, m]
    wd_perm_t = np.ascontiguousarray(wd_perm.T)                 # [m, r]
    wc_t = np.ascontiguousarray(conv_weight.T)                  # [m, o]

    xp = np.zeros((B, C, HP, WP), dtype=np.float16)
    xp[:, :, 1:1 + H, 1:1 + W] = input
    xpa = xp.reshape(B, C, LP)
    # column-shifted copy: xpb[..., j] = xpa[..., j+1] (for the kw==1 windows)
    xpb = np.zeros_like(xpa)
    xpb[:, :, :-1] = xpa[:, :, 1:]

    in_maps = []
    for i in range(NCORES):
        in_maps.append({
            "xpa": np.ascontiguousarray(xpa[i * BL:(i + 1) * BL]),
            "xpb": np.ascontiguousarray(xpb[i * BL:(i + 1) * BL]),
            "wd": wd_perm.astype(np.float16),
            "wdt": wd_perm_t,
            "wct": wc_t,
        })

    res = run_bass_kernel_spmd(nc, in_maps, core_ids=list(range(NCORES)),
                               trace=TRACE)
    _CACHE["last_result"] = res

    full = np.concatenate([res.results[i]["out"] for i in range(NCORES)], axis=0)
    return full.reshape(B, O, H, W).astype(np.float32)
